# revision 12
# baseline (speedup 1.0000x reference)
"""Trainium2 Bass kernel for HGConv (hypergraph conv) message passing, v4.

Contract: kernel(**inputs) takes FULL unsharded inputs, shards batch b
across 8 NeuronCores (data-parallel, one batch element per core), runs a
Bass/Tile kernel via run_bass_kernel_spmd, and returns the full (8, 16)
logits.

Math (per batch element), exploiting matmul associativity:
    agg  = inc^T @ nf                      # (E, D)  <- the ONLY big matmul
    es   = agg @ Wa^T                      # == inc^T @ (nf @ Wa^T)
    attn = softmax_e(es)
    ef0  = (agg * attn) @ Wp^T
    ef   = alpha * edge_feats + (1 - alpha) * ef0
    a    = softmax_e(ef @ att_w^T)
    pooled = sum_e(ef * a)
    logits = pooled @ (fc_w @ ec_proj_w)^T + (ec_proj_b @ fc_w^T + fc_b)

The kernel is PE-column-bound (fp16 streams ~2 cols/cycle ~ 5.5 Gcol/s
measured): per-pass floor = matmul output columns. v4 trims the second
softmax + pooling from 3104 columns to ~50 by computing ef in TRANSPOSED
(e-on-partitions, d-free) layout out of the ef0 matmul (x-slices as the
stationary operand), so:
  - s[e] = ef . att_w is a free-dim DVE reduction (att_w replicated
    across partitions), not an M=1 matmul (saves 2048 cols);
  - a-broadcast matmul (1024 cols) is gone: pooled = sum_e w[e]*ef[e,:]
    accumulates per-partition-scalar DVE mults (V chains) + two N=1
    matmuls for the partition sum; w = exp(s - 6 ln2) (fp16-safe V, the
    2^-6 cancels in pooled/Z);
  - Z = sum exp comes free from the exp's accum_out + one N=1 matmul.
Weight loads stay hidden: every ldweights precedes a >=128-col stream
except ~7 tiny ones (vs 16 exposed ones a direct N=1-matmul pooling
would cost).

v3: fp16 on-chip intermediates + es/ef0 at the 16-bit PE rate.
v2: fp16 HBM streams; alpha-prescaled edge feats; software-pipelined
emission: pass k-1's whole post chain interleaves into pass k's
superchunk loop with PSUM parity tags (2 passes x 2 d-halves x 4KB = all
8 banks), so the PE never waits on softmax/pool latency.

Layouts: agg/es/x are (d on partitions, e free): the first softmax's
reductions over e are free-dim. ef/eft are (e on partitions, d free):
the second softmax's per-e scalars are per-partition.
"""

import numpy as np

import concourse.bass as bass
import concourse.mybir as mybir
import concourse.tile as tile
from concourse import bacc
from concourse.bass_utils import run_bass_kernel_spmd

B, M, E, D, C = 8, 4096, 1024, 256, 16
F32 = mybir.dt.float32
F32R = mybir.dt.float32r  # full-rate matmul mode for 4-byte floats
F16 = mybir.dt.float16

NSC = 8            # superchunks in the m-loop
SUBS = 4           # 128-row chunks per superchunk


class _Pools:
    pass


def _setup(tc, aps, ctx):
    """Constant loads + pool allocation (once, outside the reps loop)."""
    nc = tc.nc
    (nf_d, inc_d, eft_d, waT_d, wpT_d, attw_d, wfT_d, bf_d, out_d) = aps

    p = _Pools()
    p.consts = ctx.enter_context(tc.tile_pool(name="consts", bufs=1))
    p.inc = ctx.enter_context(tc.tile_pool(name="inc", bufs=8))
    p.nf = ctx.enter_context(tc.tile_pool(name="nf", bufs=6))
    p.sb = ctx.enter_context(tc.tile_pool(name="sb", bufs=1))
    p.ps = ctx.enter_context(tc.tile_pool(name="ps", bufs=1, space="PSUM"))

    consts = p.consts
    p.waT = consts.tile([128, 2, D], F16, tag="waT")
    nc.sync.dma_start(p.waT[:], waT_d.rearrange("(c p) j -> p c j", p=128))
    p.wpT = consts.tile([128, 2, D], F16, tag="wpT")
    nc.sync.dma_start(p.wpT[:], wpT_d.rearrange("(c p) j -> p c j", p=128))
    # edge-attention weights replicated across all 128 partitions: (128, D)
    p.attw = consts.tile([128, D], F16, tag="attw")
    nc.sync.dma_start(p.attw[:], attw_d[:])
    p.wfT = consts.tile([128, 2, C], F32, tag="wfT")
    nc.sync.dma_start(p.wfT[:], wfT_d.rearrange("(c p) j -> p c j", p=128))
    p.bf = consts.tile([1, C], F32, tag="bf")
    nc.sync.dma_start(p.bf[:], bf_d[:])
    p.ones32 = consts.tile([128, 1], F32, tag="ones32")
    nc.gpsimd.memset(p.ones32[:], 1.0)
    p.ones16 = consts.tile([128, 1], F16, tag="ones16")
    nc.gpsimd.memset(p.ones16[:], 1.0)
    p.nln64 = consts.tile([128, 1], F32, tag="nln64")
    nc.gpsimd.memset(p.nln64[:], -6.0 * 0.6931471805599453)
    return p


def _emit_iter(tc, k, prev, p, aps, alpha):
    """Emit pass k's DMA + agg matmuls; interleave pass k-1's post chain."""
    nc = tc.nc
    (nf_d, inc_d, eft_d, *_rest) = aps
    par = k % 2

    st = {"par": par, "k": k}
    # pre-scaled (alpha) NATURAL-layout edge feats, (e-part, d) fp16:
    # tile [128, 8, D] with e = et*128 + p  (ACT HWDGE queue)
    eft = p.sb.tile([128, 8, D], F16, tag="eft", bufs=2, name=f"eft{k}")
    nc.scalar.dma_start(eft[:], eft_d.rearrange("(c p) d -> p c d", p=128))
    st["eft"] = eft

    agg = [
        p.ps.tile([128, E], F32, tag=f"ps{par}{di}", name=f"agg{k}_{di}")
        for di in range(2)
    ]
    st["agg"] = agg

    for s in range(NSC):
        rows = slice(s * SUBS * 128, (s + 1) * SUBS * 128)
        # (p c): partition p holds SUBS *contiguous* rows -> one big DMA
        # descriptor per partition (8KB for inc) instead of SUBS small ones.
        # Any m-permutation is fine: nf and inc agree, and the matmul sums
        # over the whole chunk.
        nf_t = p.nf.tile([128, SUBS, D], F16, tag="nf", name=f"nf{k}_{s}")
        nc.scalar.dma_start(nf_t[:], nf_d[rows, :].rearrange("(p c) d -> p c d", p=128))
        inc_t = p.inc.tile([128, SUBS, E], F16, tag="inc", name=f"inc{k}_{s}")
        inc_src = inc_d[rows, :].rearrange("(p c) e -> p c e", p=128)
        nc.sync.dma_start(inc_t[:, 0:SUBS // 2], inc_src[:, 0:SUBS // 2, :])
        nc.gpsimd.dma_start(inc_t[:, SUBS // 2:SUBS], inc_src[:, SUBS // 2:SUBS, :])
        for c in range(SUBS):
            first = s == 0 and c == 0
            last = s == NSC - 1 and c == SUBS - 1
            for di in range(2):
                lhsT = nf_t[:, c, di * 128:(di + 1) * 128]
                for eh in range(2):
                    nc.tensor.matmul(
                        agg[di][:, eh * 512:(eh + 1) * 512],
                        lhsT,
                        inc_t[:, c, eh * 512:(eh + 1) * 512],
                        start=first,
                        stop=last,
                    )
        if prev is not None:
            _post_stage(tc, s, prev, p, aps, alpha)
    return st


def _post_stage(tc, s, st, p, aps, alpha):
    """Stage s (0..7) of the post-aggregation chain for the pass in `st`."""
    nc = tc.nc
    out_d = aps[-1]
    par = st["par"]
    k = st["k"]
    sb, ps = p.sb, p.ps

    if s == 0:
        # PSUM -> SBUF copy of agg (frees nothing yet; es will reuse banks)
        st["agg_sb"] = [
            sb.tile([128, E], F16, tag=f"aggsb{di}", name=f"aggsb{k}_{di}")
            for di in range(2)
        ]
        for eh in range(2):
            ehs = slice(eh * 512, (eh + 1) * 512)
            nc.vector.tensor_copy(st["agg_sb"][0][:, ehs], st["agg"][0][:, ehs])
            nc.scalar.mul(st["agg_sb"][1][:, ehs], st["agg"][1][:, ehs], 1.0)

    elif s == 1:
        # es = Wa @ agg (PSUM banks of this pass's parity, now WAR-free)
        es = [
            ps.tile([128, E], F32, tag=f"ps{par}{di}", name=f"es{k}_{di}")
            for di in range(2)
        ]
        st["es"] = es
        for di in range(2):
            for dk in range(2):
                lhsT = p.waT[:, dk, di * 128:(di + 1) * 128]
                for eh in range(2):
                    nc.tensor.matmul(
                        es[di][:, eh * 512:(eh + 1) * 512],
                        lhsT,
                        st["agg_sb"][dk][:, eh * 512:(eh + 1) * 512],
                        start=dk == 0,
                        stop=dk == 1,
                    )
        # softmax over e (free dim): exp(es - max), then X = attn * agg
        st["x"] = []
        for di in range(2):
            nmax = sb.tile([128, 1], F32, tag=f"nmax{di}", name=f"nmax{k}_{di}")
            nc.vector.tensor_reduce(nmax[:], es[di][:], axis=mybir.AxisListType.X,
                                    op=mybir.AluOpType.max, negate=True)
            expt = sb.tile([128, E], F16, tag=f"exp{di}", name=f"exp{k}_{di}")
            rsum = sb.tile([128, 1], F32, tag=f"rsum{di}", name=f"rsum{k}_{di}")
            nc.scalar.activation(expt[:], es[di][:],
                                 mybir.ActivationFunctionType.Exp,
                                 bias=nmax[:], accum_out=rsum[:])
            rinv = sb.tile([128, 1], F32, tag=f"rinv{di}", name=f"rinv{k}_{di}")
            nc.vector.reciprocal(rinv[:], rsum[:])
            xt = sb.tile([128, E], F16, tag=f"x{di}", name=f"x{k}_{di}")
            nc.vector.scalar_tensor_tensor(xt[:], expt[:], rinv[:],
                                           st["agg_sb"][di][:],
                                           op0=mybir.AluOpType.mult,
                                           op1=mybir.AluOpType.mult)
            st["x"].append(xt)

    elif s == 3:
        # ef0T = X^T @ Wp^T in TRANSPOSED (e-part, d-free) layout:
        # out[et][128e, D] = sum_dk x[dk][:, et-slice]^T @ wpT[dk]
        # Packed 4 et per parity tag: [128, 4, D] f32 = 4KB = 2 banks.
        ef0t = [
            ps.tile([128, 4, D], F32, tag=f"ps{par}{di}", name=f"ef0t{k}_{di}")
            for di in range(2)
        ]
        st["ef0t"] = ef0t
        for half in range(2):
            for j in range(4):
                et = half * 4 + j
                for dk in range(2):
                    nc.tensor.matmul(
                        ef0t[half][:, j, :],
                        st["x"][dk][:, et * 128:(et + 1) * 128],
                        p.wpT[:, dk, :],
                        start=dk == 0,
                        stop=dk == 1,
                    )

    elif s == 4:
        # blend: ef_et = (1-alpha)*ef0T_et + eftn_et   (eft alpha-prescaled)
        st["ef"] = []
        for et in range(8):
            ef_et = sb.tile([128, D], F16, tag=f"ef{et}", name=f"ef{k}_{et}")
            nc.vector.scalar_tensor_tensor(ef_et[:],
                                           st["ef0t"][et // 4][:, et % 4, :],
                                           1.0 - alpha, st["eft"][:, et, :],
                                           op0=mybir.AluOpType.mult,
                                           op1=mybir.AluOpType.add)
            st["ef"].append(ef_et)

    elif s == 5:
        # edge attention scores s[e] = sum_d ef[e, d] * att_w[d]:
        # free-dim reduce per e-tile (DVE), then one exp over [128, 8]
        scols = sb.tile([128, 8], F32, tag="scols", name=f"scols{k}")
        for et in range(8):
            junk = sb.tile([128, D], F16, tag=f"sjunk{et % 2}",
                           name=f"sjunk{k}_{et}")
            nc.vector.scalar_tensor_tensor(junk[:], st["ef"][et][:], 1.0,
                                           p.attw[:],
                                           op0=mybir.AluOpType.mult,
                                           op1=mybir.AluOpType.mult,
                                           accum_out=scols[:, et:et + 1])
        # |s| <= ~7 for this model: softmax safe without max-subtraction.
        # w = exp(s)*2^-6 (bias = -6*ln2) so the fp16 V accumulation below
        # cannot overflow; Z picks up the same 2^-6 via accum_out, so the
        # final pooled*(1/Z) ratio is unchanged.
        # accum_out over the 8 free elems gives per-partition partial Z sums.
        w = sb.tile([128, 8], F32, tag="w", name=f"w{k}")
        zpart = sb.tile([128, 1], F32, tag="zpart", name=f"zpart{k}")
        nc.scalar.activation(w[:], scols[:], mybir.ActivationFunctionType.Exp,
                             bias=p.nln64[:], accum_out=zpart[:])
        st["w"] = w
        st["zpart"] = zpart

    elif s == 6:
        # V[ci] = sum_{et in chain ci} ef_et * w_et (per-partition scalar
        # mults on DVE, two independent chains), then pooledraw[d] =
        # sum_p V[ci][p, d] via two N=1 matmul accumulations per di.
        w = st["w"]
        vfin = []
        for ci in range(2):
            v_prev = None
            for j in range(4):
                et = ci * 4 + j
                v_new = sb.tile([128, D], F16, tag=f"V{ci}{j % 2}",
                                name=f"V{k}_{ci}_{j}")
                if v_prev is None:
                    nc.vector.tensor_scalar_mul(v_new[:], st["ef"][et][:],
                                                w[:, et:et + 1])
                else:
                    nc.vector.scalar_tensor_tensor(v_new[:], st["ef"][et][:],
                                                   w[:, et:et + 1], v_prev[:],
                                                   op0=mybir.AluOpType.mult,
                                                   op1=mybir.AluOpType.add)
                v_prev = v_new
            vfin.append(v_prev)
        pooled_ps = [
            ps.tile([128, 1], F32, tag=f"ps{par}{di}", name=f"pool{k}_{di}")
            for di in range(2)
        ]
        for di in range(2):
            for ci in range(2):
                nc.tensor.matmul(
                    pooled_ps[di][:],
                    vfin[ci][:, di * 128:(di + 1) * 128],
                    p.ones16[:],
                    start=ci == 0,
                    stop=ci == 1,
                )
        st["pooled"] = []
        for di, eng in zip(range(2), (nc.vector, nc.scalar)):
            pcol = sb.tile([128, 1], F32, tag=f"pooled{di}", name=f"pl{k}_{di}")
            if di == 0:
                eng.tensor_copy(pcol[:], pooled_ps[di][:])
            else:
                eng.mul(pcol[:], pooled_ps[di][:], 1.0)
            st["pooled"].append(pcol)

    elif s == 7:
        # Z = sum_p zpart (fp32 K=128, N=1 matmul), logits =
        # (pooledraw @ WfoldT) * (1/Z) + bfold
        z_ps = ps.tile([1, 1], F32, tag=f"ps{par}0", name=f"z{k}")
        nc.tensor.matmul(z_ps[:], st["zpart"][:], p.ones32[:],
                         start=True, stop=True)
        zinv = sb.tile([1, 1], F32, tag="zinv", name=f"zinv{k}")
        nc.vector.reciprocal(zinv[:], z_ps[:])
        lg_ps = ps.tile([1, C], F32, tag=f"ps{par}1", name=f"lg{k}")
        for dk in range(2):
            nc.tensor.matmul(lg_ps[:], st["pooled"][dk][:], p.wfT[:, dk, :],
                             start=dk == 0, stop=dk == 1)
        lg_sb = sb.tile([1, C], F32, tag="lgsb", name=f"lgsb{k}")
        nc.vector.scalar_tensor_tensor(lg_sb[:], lg_ps[:], zinv[:], p.bf[:],
                                       op0=mybir.AluOpType.mult,
                                       op1=mybir.AluOpType.add)
        nc.sync.dma_start(out_d[:], lg_sb[:])


def _kernel_body(tc, aps, alpha: float, ctx, reps: int = 1):
    p = _setup(tc, aps, ctx)
    prev = None
    for k in range(reps):
        prev = _emit_iter(tc, k, prev, p, aps, alpha)
    for s in range(NSC):  # drain the last pass's chain
        _post_stage(tc, s, prev, p, aps, alpha)


def build(alpha: float, reps: int = 1):
    nc = bacc.Bacc("TRN2", target_bir_lowering=False, debug=False)
    nf_d = nc.dram_tensor("node_feats", [M, D], F16, kind="ExternalInput").ap()
    inc_d = nc.dram_tensor("inc_mat", [M, E], F16, kind="ExternalInput").ap()
    eft_d = nc.dram_tensor("eftn", [E, D], F16, kind="ExternalInput").ap()
    waT_d = nc.dram_tensor("waT", [D, D], F16, kind="ExternalInput").ap()
    wpT_d = nc.dram_tensor("wpT", [D, D], F16, kind="ExternalInput").ap()
    attw_d = nc.dram_tensor("attw", [128, D], F16, kind="ExternalInput").ap()
    wfT_d = nc.dram_tensor("wfoldT", [D, C], F32, kind="ExternalInput").ap()
    bf_d = nc.dram_tensor("bfold", [1, C], F32, kind="ExternalInput").ap()
    out_d = nc.dram_tensor("logits", [1, C], F32, kind="ExternalOutput").ap()
    aps = (nf_d, inc_d, eft_d, waT_d, wpT_d, attw_d, wfT_d, bf_d, out_d)
    from contextlib import ExitStack

    with tile.TileContext(nc) as tc, ExitStack() as ctx:
        _kernel_body(tc, aps, alpha, ctx, reps=reps)
    nc.compile()
    return nc


def make_in_maps(inputs: dict) -> list[dict]:
    nf = np.asarray(inputs["node_feats"], np.float32)
    inc = np.asarray(inputs["inc_mat"], np.float32)
    ef = np.asarray(inputs["edge_feats"], np.float32)
    alpha = float(np.asarray(inputs["alpha"]))
    Wa = np.asarray(inputs["Wa"], np.float32)
    Wp = np.asarray(inputs["Wp"], np.float32)
    att = np.asarray(inputs["ec_att_w"], np.float32).reshape(1, D)
    ec_w = np.asarray(inputs["ec_proj_w"], np.float32)
    ec_b = np.asarray(inputs["ec_proj_b"], np.float32)
    fc_w = np.asarray(inputs["fc_w"], np.float32)
    fc_b = np.asarray(inputs["fc_b"], np.float32)

    waT = np.ascontiguousarray(Wa.T.astype(np.float16))
    wpT = np.ascontiguousarray(Wp.T.astype(np.float16))
    # replicate edge-attention weights across 128 partitions: (128, D)
    attw = np.ascontiguousarray(
        np.broadcast_to(att.astype(np.float16), (128, D)))
    wfoldT = np.ascontiguousarray((fc_w @ ec_w).T)          # (D, C)
    bfold = np.ascontiguousarray((ec_b @ fc_w.T + fc_b).reshape(1, C))

    nf16 = np.ascontiguousarray(nf.astype(np.float16))
    inc16 = np.ascontiguousarray(inc.astype(np.float16))
    # alpha-scale edge feats host-side, NATURAL (E, D) layout fp16
    eft16 = np.ascontiguousarray((alpha * ef).astype(np.float16))

    return [
        dict(node_feats=nf16[b], inc_mat=inc16[b], eftn=eft16[b],
             waT=waT, wpT=wpT, attw=attw, wfoldT=wfoldT, bfold=bfold)
        for b in range(B)
    ]


def kernel(**inputs) -> np.ndarray:
    alpha = float(np.asarray(inputs["alpha"]))
    nc = build(alpha)
    in_maps = make_in_maps(inputs)
    res = run_bass_kernel_spmd(nc, in_maps, core_ids=list(range(B)))
    return np.stack([res.results[b]["logits"].reshape(C) for b in range(B)], axis=0)

